# revision 1
# baseline (speedup 1.0000x reference)
"""Neighborhood attention (7x7) Trainium2 Bass kernel.

Sharding: 8 cores = 4 batches x 2 row-halves (32 rows each).
Per core: project q/k/v from a zero-padded 40-row slice, banded attention
via dense Gram matmuls in S^T orientation ([key_px, query_px] -- no
transposes anywhere), masked exp, Z via M=1 packed matmuls, AV via
col-packed form-B v, out-projection. All biases folded on host:
  - SCALE folded into Wq
  - bk's score term is per-query constant -> cancels in softmax (dropped)
  - bq's term u_h[key] = SCALE*bq_h.(Wk_h x) enters as exp(u) folded
    multiplicatively into the AV lhsT (v' = v*expu) and the Z lhsT
  - bv folded into the output bias: bo' = wo@bv + bo
"""
import sys
import numpy as np
from contextlib import ExitStack

sys.path.insert(0, "/opt/trn_rl_repo")

import concourse.bass as bass
import concourse.bacc as bacc
import concourse.mybir as mybir
import concourse.tile as tile
from concourse.bass_utils import run_bass_kernel_spmd

DIM, HEADS, HD = 256, 8, 32
SCALE = HD ** -0.5
B, H, W = 4, 64, 64
KVR = 40          # kv rows per core (zero padded)
NKB = KVR // 2    # 20 kblocks of 2 rows (128 px)
NPX = KVR * 64    # 2560 kv pixels
QOFF = 4 * 64     # own-query offset inside kv pixels
NQ = 2048         # own query pixels (32 rows)
F32 = mybir.dt.float32


def _core_geom(core):
    b, half = core // 2, core % 2
    return b, half * 32  # batch, R0


def _qcol0(kb):
    return 64 * min(max(2 * kb - 7, 0), 24)


def _contribs():
    """region -> (list of kb, first, last)"""
    out = []
    for r in range(4):
        kbs = [kb for kb in range(NKB)
               if _qcol0(kb) < 512 * r + 512 and _qcol0(kb) + 512 > 512 * r]
        out.append(kbs)
    return out


def _build_mask(R0, kb):
    """[128 kpx, 512 qwin] in {0,1}, window rows = R0 + lo_rel .. +8"""
    krow = R0 - 4 + 2 * kb
    lo_r = R0 + _qcol0(kb) // 64
    kp = np.arange(128)
    rk = krow + kp // 64
    ck = kp % 64
    qc = np.arange(512)
    rq = lo_r + qc // 64
    cq = qc % 64
    ok_k = ((rk >= 0) & (rk < H))[:, None]
    band = (np.abs(rk[:, None] - rq[None, :]) <= 3) & \
           (np.abs(ck[:, None] - cq[None, :]) <= 3)
    return (ok_k & band).astype(np.float32)


DEBUG = False


def _build_program():
    nc = bacc.Bacc(trn_type="TRN2", target_bir_lowering=False, debug=False,
                   num_devices=8)
    d = {}
    d["x"] = nc.dram_tensor("x", [DIM, NPX], F32, kind="ExternalInput")
    for w in ["wqT", "wkT", "wvT", "woT"]:
        d[w] = nc.dram_tensor(w, [DIM, DIM], F32, kind="ExternalInput")
    d["wu"] = nc.dram_tensor("wu", [DIM, HEADS], F32, kind="ExternalInput")
    d["masks"] = nc.dram_tensor("masks", [128, NKB * 512], F32,
                                kind="ExternalInput")
    d["repmat"] = nc.dram_tensor("repmat", [128, 128], F32,
                                 kind="ExternalInput")
    d["bo2"] = nc.dram_tensor("bo2", [128, 2], F32, kind="ExternalInput")
    y = nc.dram_tensor("y", [DIM, NQ], F32, kind="ExternalOutput")
    if DEBUG:
        dbg = {n: nc.dram_tensor(n, s, F32, kind="ExternalOutput") for n, s in [
            ("dq", [DIM, NQ]), ("dk", [DIM, NPX]), ("dv", [128, NKB * 256]),
            ("deu", [128, NKB * 8]), ("dz", [128, 2 * NQ]),
            ("davn", [128, 2 * NQ]), ("dmsk", [128, 2048]), ("dexp", [128, 2048])]}

    contribs = _contribs()
    first_kb = [min(k) for k in contribs]
    last_kb = [max(k) for k in contribs]

    with ExitStack() as ctx:
        tc = ctx.enter_context(tile.TileContext(nc))
        cp = ctx.enter_context(tc.tile_pool(name="const", bufs=1))
        sp = ctx.enter_context(tc.tile_pool(name="spsum", bufs=1, space="PSUM"))
        zp = ctx.enter_context(tc.tile_pool(name="zpsum", bufs=1, space="PSUM"))
        ap = ctx.enter_context(tc.tile_pool(name="avpsum", bufs=2, space="PSUM"))
        wp = ctx.enter_context(tc.tile_pool(name="work", bufs=2))

        # ---- load constants / inputs ----
        x_sb = cp.tile([128, 2 * NPX], F32)
        for kt in range(2):
            nc.sync.dma_start(x_sb[:, NPX * kt:NPX * (kt + 1)],
                              d["x"][128 * kt:128 * (kt + 1), :])
        w_sb = {}
        for w in ["wqT", "wkT", "wvT", "woT"]:
            w_sb[w] = cp.tile([128, 512], F32, tag=w, name=w)
            for kt in range(2):
                nc.sync.dma_start(w_sb[w][:, 256 * kt:256 * (kt + 1)],
                                  d[w][128 * kt:128 * (kt + 1), :])
        wu_sb = cp.tile([128, 16], F32)
        for kt in range(2):
            nc.sync.dma_start(wu_sb[:, 8 * kt:8 * (kt + 1)],
                              d["wu"][128 * kt:128 * (kt + 1), :])
        masks_sb = cp.tile([128, NKB * 512], F32)
        nc.sync.dma_start(masks_sb[:], d["masks"][:])
        repmat_sb = cp.tile([128, 128], F32)
        nc.sync.dma_start(repmat_sb[:], d["repmat"][:])
        bo2_sb = cp.tile([128, 2], F32)
        nc.sync.dma_start(bo2_sb[:], d["bo2"][:])

        q_sb = [cp.tile([128, NQ], F32, tag=f"q{m}", name=f"q{m}") for m in range(2)]
        k_sb = [cp.tile([128, NPX], F32, tag=f"k{m}", name=f"k{m}") for m in range(2)]
        expu_sb = cp.tile([128, NKB * 8], F32)
        z_sb = cp.tile([128, 2 * NQ], F32)       # rows {0,32,64,96} valid
        avn_sb = cp.tile([128, 2 * NQ], F32)
        zrow_sb = cp.tile([1, 512], F32)
        nc.vector.memset(z_sb[:], 1e-30)
        nc.vector.memset(zrow_sb[:], 0.0)

        # ---- q / k natural projections ----
        for m in range(2):
            nn = NQ // 512
            for n in range(nn):
                ps = ap.tile([128, 512], F32, tag="av", name=f"psq{m}_{n}")
                for kt in range(2):
                    nc.tensor.matmul(
                        ps[:],
                        w_sb["wqT"][:, 256 * kt + 128 * m:256 * kt + 128 * m + 128],
                        x_sb[:, NPX * kt + QOFF + 512 * n:
                             NPX * kt + QOFF + 512 * n + 512],
                        start=(kt == 0), stop=(kt == 1))
                nc.vector.tensor_copy(q_sb[m][:, 512 * n:512 * n + 512], ps[:])
            for n in range(NPX // 512):
                ps = ap.tile([128, 512], F32, tag="av", name=f"psk{m}_{n}")
                for kt in range(2):
                    nc.tensor.matmul(
                        ps[:],
                        w_sb["wkT"][:, 256 * kt + 128 * m:256 * kt + 128 * m + 128],
                        x_sb[:, NPX * kt + 512 * n:NPX * kt + 512 * n + 512],
                        start=(kt == 0), stop=(kt == 1))
                nc.vector.tensor_copy(k_sb[m][:, 512 * n:512 * n + 512], ps[:])

        # ---- v + u form-B projections (per kblock px tile) ----
        v_raw = cp.tile([128, NKB * 256], F32)
        for t in range(NKB):
            ps = ap.tile([128, 264], F32, tag="av", name=f"psvu{t}")
            for kt in range(2):
                lhsT = x_sb[:, NPX * kt + 128 * t:NPX * kt + 128 * t + 128]
                nc.tensor.matmul(ps[:, 0:256],
                                 lhsT, w_sb["wvT"][:, 256 * kt:256 * kt + 256],
                                 start=(kt == 0), stop=False,
                                 skip_group_check=True)
            for kt in range(2):
                lhsT = x_sb[:, NPX * kt + 128 * t:NPX * kt + 128 * t + 128]
                nc.tensor.matmul(ps[:, 256:264],
                                 lhsT, wu_sb[:, 8 * kt:8 * kt + 8],
                                 start=False, stop=(kt == 1),
                                 skip_group_check=True)
            nc.scalar.activation(expu_sb[:, 8 * t:8 * t + 8], ps[:, 256:264],
                                 mybir.ActivationFunctionType.Exp)
            eu = expu_sb[:]
            pa = ps[:]
            v_out = bass.AP(v_raw[:].tensor, v_raw[:].offset + 256 * t,
                            [[NKB * 256, 128], [32, 8], [1, 32]])
            v_in = bass.AP(pa.tensor, pa.offset, [[264, 128], [32, 8], [1, 32]])
            eu_in = bass.AP(eu.tensor, eu.offset + 8 * t,
                            [[NKB * 8, 128], [1, 8], [0, 32]])
            nc.vector.tensor_mul(v_out, v_in, eu_in)
        v2_sb = v_raw

        # ---- attention: g-outermost ----
        msk = masks_sb[:]
        for g in range(2):
            for kb in range(NKB):
                qc0 = _qcol0(kb)
                spsum = sp.tile([128, 2048], F32, tag="s")
                for hh in range(4):
                    nc.tensor.matmul(
                        spsum[:, 512 * hh:512 * hh + 512],
                        k_sb[g][32 * hh:32 * hh + 32, 128 * kb:128 * kb + 128],
                        q_sb[g][32 * hh:32 * hh + 32, qc0:qc0 + 512],
                        start=True, stop=True, tile_position=(32 * hh, 0))
                exp_t = wp.tile([128, 2048], F32, tag="exp")
                nc.scalar.activation(exp_t[:], spsum[:],
                                     mybir.ActivationFunctionType.Exp)
                msk_t = wp.tile([128, 2048], F32, tag="msk")
                mask_bcast = bass.AP(msk.tensor, msk.offset + 512 * kb,
                                     [[NKB * 512, 128], [0, 4], [1, 512]])
                eng = nc.gpsimd if kb % 4 == 3 else nc.vector
                eng.tensor_mul(msk_t[:], exp_t[:], mask_bcast)
                if DEBUG and g == 0 and kb == 10:
                    nc.sync.dma_start(dbg["dexp"][:], exp_t[:])
                    nc.sync.dma_start(dbg["dmsk"][:], msk_t[:])
                # Z: M=1 matmuls, lhsT = expu column (folds u in)
                zps = zp.tile([128, 512], F32, tag="z")
                for hh in range(4):
                    nc.tensor.matmul(
                        zps[32 * hh:32 * hh + 1, 0:512],
                        expu_sb[:, 8 * kb + 4 * g + hh:8 * kb + 4 * g + hh + 1],
                        msk_t[:, 512 * hh:512 * hh + 512],
                        start=True, stop=True, tile_position=(0, 32 * hh))
                zeng = nc.vector
                zsl = z_sb[:, NQ * g + qc0:NQ * g + qc0 + 512]
                zeng.tensor_add(zsl, zsl, zps[:])
                # AV accumulate into region psums
                for r in range(4):
                    if kb not in contribs[r]:
                        continue
                    avp = _get_av(ap, g, r)
                    if kb == first_kb[r]:
                        nc.tensor.matmul(avp[:], zrow_sb[0:1, 0:128],
                                         zrow_sb[0:1, 0:512],
                                         start=True, stop=False)
                    c0 = max(512 * r, qc0)
                    c1 = min(512 * r + 512, qc0 + 512)
                    for hh in range(4):
                        nc.tensor.matmul(
                            avp[32 * hh:32 * hh + 32, c0 - 512 * r:c1 - 512 * r],
                            v2_sb[:, 256 * kb + 32 * (4 * g + hh):
                                  256 * kb + 32 * (4 * g + hh) + 32],
                            msk_t[:, 512 * hh + c0 - qc0:512 * hh + c1 - qc0],
                            start=False, stop=(kb == last_kb[r] and hh == 3),
                            tile_position=(0, 32 * hh))
                    if kb == last_kb[r]:
                        # finalize region r for group g
                        zr_t = wp.tile([128, 512], F32, tag="zr")
                        nc.vector.reciprocal(
                            zr_t[:], z_sb[:, NQ * g + 512 * r:NQ * g + 512 * r + 512])
                        zrp = zp.tile([128, 512], F32, tag="z")
                        nc.tensor.matmul(zrp[:], repmat_sb[:], zr_t[:],
                                         start=True, stop=True)
                        zrep_t = wp.tile([128, 512], F32, tag="zrep")
                        nc.vector.tensor_copy(zrep_t[:], zrp[:])
                        nc.vector.tensor_mul(
                            avn_sb[:, NQ * g + 512 * r:NQ * g + 512 * r + 512],
                            avp[:], zrep_t[:])
                        _free_av(g, r)

        if DEBUG:
            for m in range(2):
                nc.sync.dma_start(dbg["dq"][128 * m:128 * m + 128, :], q_sb[m][:])
                nc.sync.dma_start(dbg["dk"][128 * m:128 * m + 128, :], k_sb[m][:])
            nc.sync.dma_start(dbg["dv"][:], v2_sb[:])
            nc.sync.dma_start(dbg["deu"][:], expu_sb[:])
            nc.sync.dma_start(dbg["dz"][:], z_sb[:])
            nc.sync.dma_start(dbg["davn"][:], avn_sb[:])

        # ---- output projection ----
        for r in range(4):
            for m in range(2):
                ps = ap.tile([128, 512], F32, tag="av", name=f"psop{r}_{m}")
                for g in range(2):
                    nc.tensor.matmul(
                        ps[:],
                        w_sb["woT"][:, 256 * g + 128 * m:256 * g + 128 * m + 128],
                        avn_sb[:, NQ * g + 512 * r:NQ * g + 512 * r + 512],
                        start=(g == 0), stop=(g == 1))
                ot = wp.tile([128, 512], F32, tag="zr", name=f"ot{r}_{m}")
                nc.vector.tensor_scalar_add(ot[:], ps[:], bo2_sb[:, m:m + 1])
                nc.sync.dma_start(y[128 * m:128 * m + 128, 512 * r:512 * r + 512],
                                  ot[:])

    nc.compile()
    return nc


_av_tiles = {}


def _get_av(pool, g, r):
    key = (g, r)
    if key not in _av_tiles:
        _av_tiles[key] = pool.tile([128, 512], F32, tag="av", name=f"av{g}_{r}")
    return _av_tiles[key][:]


def _free_av(g, r):
    _av_tiles.pop((g, r), None)


def _host_inputs(inputs):
    query = np.ascontiguousarray(inputs["query"], np.float32)
    wq = np.asarray(inputs["wq"], np.float32)
    bq = np.asarray(inputs["bq"], np.float32)
    wk = np.asarray(inputs["wk"], np.float32)
    wv = np.asarray(inputs["wv"], np.float32)
    bv = np.asarray(inputs["bv"], np.float32)
    wo = np.asarray(inputs["wo"], np.float32)
    bo = np.asarray(inputs["bo"], np.float32)

    wqT = np.ascontiguousarray((wq * SCALE).T)
    wkT = np.ascontiguousarray(wk.T)
    wvT = np.ascontiguousarray(wv.T)
    woT = np.ascontiguousarray(wo.T)
    wu = np.stack([SCALE * (wk[32 * h:32 * h + 32].T @ bq[32 * h:32 * h + 32])
                   for h in range(HEADS)], axis=1).astype(np.float32)
    bo2v = (wo @ bv + bo).astype(np.float32)
    bo2 = np.ascontiguousarray(bo2v.reshape(2, 128).T)  # [128, 2] col m
    repmat = np.zeros((128, 128), np.float32)
    for pout in range(128):
        repmat[32 * (pout // 32), pout] = 1.0

    in_maps = []
    for core in range(8):
        b, R0 = _core_geom(core)
        xpad = np.zeros((DIM, KVR, 64), np.float32)
        lo, hi = R0 - 4, R0 + 36
        slo, shi = max(lo, 0), min(hi, H)
        xpad[:, slo - lo:shi - lo, :] = query[b][:, slo:shi, :]
        masks = np.concatenate([_build_mask(R0, kb) for kb in range(NKB)],
                               axis=1)
        in_maps.append({
            "x": np.ascontiguousarray(xpad.reshape(DIM, NPX)),
            "wqT": wqT, "wkT": wkT, "wvT": wvT, "woT": woT, "wu": wu,
            "masks": np.ascontiguousarray(masks),
            "repmat": repmat, "bo2": bo2,
        })
    return in_maps


_nc_cache = None


def kernel(**inputs):
    global _nc_cache, _av_tiles
    in_maps = _host_inputs(inputs)
    if _nc_cache is None:
        _av_tiles = {}
        _nc_cache = _build_program()
    res = run_bass_kernel_spmd(_nc_cache, in_maps, core_ids=list(range(8)))
    out = np.zeros((B, DIM, H, W), np.float32)
    for core in range(8):
        b, R0 = _core_geom(core)
        out[b][:, R0:R0 + 32, :] = res.results[core]["y"].reshape(DIM, 32, 64)
    return out



# revision 3
# speedup vs baseline: 21263.7523x; 21263.7523x over previous
"""Neighborhood attention (7x7) Trainium2 Bass kernel.

Sharding: 8 cores = 4 batches x 2 row-halves (32 rows each).
Per core: project q/k/v from a zero-padded 40-row slice, banded attention
via dense Gram matmuls in S^T orientation ([key_px, query_px]), masked exp,
Z and AV accumulated in PSUM region tiles, out-projection.

V2 speedups over the original:
  - all big matmuls in float32r (1 cyc/row vs 4 for fp32) or bf16
  - bq folded into the q projection bias (per-partition add) instead of
    the exp(u) multiplicative machinery; Z is a ones-column matmul
  - Z accumulates in PSUM across kblocks (no DVE adds)
  - exp writes bf16, mask-mul runs in DVE 4x mode, masks shipped as bf16
  - software-pipelined emission: exp/mask(kb-1) -> S(kb) -> Z/AV(kb-1)
"""
import sys
import numpy as np
from contextlib import ExitStack

sys.path.insert(0, "/opt/trn_rl_repo")

import concourse.bass as bass
import concourse.bacc as bacc
import concourse.mybir as mybir
import concourse.tile as tile
from concourse.bass_utils import run_bass_kernel_spmd

import ml_dtypes

DIM, HEADS, HD = 256, 8, 32
SCALE = HD ** -0.5
B, H, W = 4, 64, 64
KVR = 40          # kv rows per core (zero padded)
NKB = KVR // 2    # 20 kblocks of 2 rows (128 px)
NPX = KVR * 64    # 2560 kv pixels
QOFF = 4 * 64     # own-query offset inside kv pixels
NQ = 2048         # own query pixels (32 rows)
F32 = mybir.dt.float32
F32R = mybir.dt.float32r
BF16 = mybir.dt.bfloat16


def _core_geom(core):
    b, half = core // 2, core % 2
    return b, half * 32  # batch, R0


def _qcol0(kb):
    return 64 * min(max(2 * kb - 7, 0), 24)


def _contribs():
    """region -> list of kb contributing"""
    out = []
    for r in range(4):
        kbs = [kb for kb in range(NKB)
               if _qcol0(kb) < 512 * r + 512 and _qcol0(kb) + 512 > 512 * r]
        out.append(kbs)
    return out


def _build_mask(R0, kb):
    """[128 kpx, 512 qwin] in {0,1}, window rows = R0 + lo_rel .. +8"""
    krow = R0 - 4 + 2 * kb
    lo_r = R0 + _qcol0(kb) // 64
    kp = np.arange(128)
    rk = krow + kp // 64
    ck = kp % 64
    qc = np.arange(512)
    rq = lo_r + qc // 64
    cq = qc % 64
    ok_k = ((rk >= 0) & (rk < H))[:, None]
    band = (np.abs(rk[:, None] - rq[None, :]) <= 3) & \
           (np.abs(ck[:, None] - cq[None, :]) <= 3)
    return (ok_k & band).astype(np.float32)


def _r(ap):
    return ap.bitcast(F32R)


def _build_program():
    nc = bacc.Bacc(trn_type="TRN2", target_bir_lowering=False, debug=False,
                   num_devices=8)
    d = {}
    d["x"] = nc.dram_tensor("x", [DIM, NPX], F32, kind="ExternalInput")
    for w in ["wqT", "wkT", "wvT", "woT"]:
        d[w] = nc.dram_tensor(w, [DIM, DIM], F32, kind="ExternalInput")
    d["masks"] = nc.dram_tensor("masks", [128, NKB * 512], BF16,
                                kind="ExternalInput")
    d["repmat"] = nc.dram_tensor("repmat", [128, 128], F32,
                                 kind="ExternalInput")
    d["bq2"] = nc.dram_tensor("bq2", [128, 2], F32, kind="ExternalInput")
    d["bo2"] = nc.dram_tensor("bo2", [128, 2], F32, kind="ExternalInput")
    y = nc.dram_tensor("y", [DIM, NQ], F32, kind="ExternalOutput")

    contribs = _contribs()
    first_kb = [min(k) for k in contribs]
    last_kb = [max(k) for k in contribs]

    with ExitStack() as ctx:
        tc = ctx.enter_context(tile.TileContext(nc))
        cp = ctx.enter_context(tc.tile_pool(name="const", bufs=1))
        sp = ctx.enter_context(tc.tile_pool(name="spsum", bufs=2, space="PSUM"))
        avp = ctx.enter_context(tc.tile_pool(name="avpsum", bufs=2, space="PSUM"))
        zp = ctx.enter_context(tc.tile_pool(name="zpsum", bufs=2, space="PSUM"))
        wp = ctx.enter_context(tc.tile_pool(name="work", bufs=3))

        # ---- load constants / inputs ----
        x_sb = cp.tile([128, 2 * NPX], F32)
        for kt in range(2):
            nc.sync.dma_start(x_sb[:, NPX * kt:NPX * (kt + 1)],
                              d["x"][128 * kt:128 * (kt + 1), :])
        w_sb = {}
        for w in ["wqT", "wkT", "wvT", "woT"]:
            w_sb[w] = cp.tile([128, 512], F32, tag=w, name=w)
            for kt in range(2):
                nc.sync.dma_start(w_sb[w][:, 256 * kt:256 * (kt + 1)],
                                  d[w][128 * kt:128 * (kt + 1), :])
        masks_sb = cp.tile([128, NKB * 512], BF16)
        nc.sync.dma_start(masks_sb[:], d["masks"][:])
        repmat_sb = cp.tile([128, 128], F32)
        nc.sync.dma_start(repmat_sb[:], d["repmat"][:])
        bq2_sb = cp.tile([128, 2], F32)
        nc.sync.dma_start(bq2_sb[:], d["bq2"][:])
        bo2_sb = cp.tile([128, 2], F32)
        nc.sync.dma_start(bo2_sb[:], d["bo2"][:])

        ones_sb = cp.tile([128, 1], BF16)
        nc.vector.memset(ones_sb[:], 1.0)
        zrow_sb = cp.tile([1, 512], BF16)
        nc.vector.memset(zrow_sb[:], 0.0)
        onerow_sb = cp.tile([1, 512], BF16)
        nc.vector.memset(onerow_sb[:], 1.0)
        epsrow_sb = cp.tile([1, 512], BF16)
        nc.vector.memset(epsrow_sb[:], 1e-30)

        q_sb = [cp.tile([128, NQ], F32, tag=f"q{m}", name=f"q{m}") for m in range(2)]
        k_sb = [cp.tile([128, NPX], F32, tag=f"k{m}", name=f"k{m}") for m in range(2)]
        avn_sb = cp.tile([128, 2 * NQ], F32)
        v2_sb = cp.tile([128, NKB * 256], BF16)

        # ---- projections (q, k natural; v form-B) ----
        ppools = [(sp, "s"), (avp, "av"), (zp, "z")]
        pidx = 0

        def _ptile(shape, name):
            nonlocal pidx
            p, tag = ppools[pidx % 3]
            pidx += 1
            return p.tile(shape, F32, tag=tag, name=name)

        for m in range(2):
            for n in range(NQ // 512):
                ps = _ptile([128, 512], f"psq{m}_{n}")
                for kt in range(2):
                    nc.tensor.matmul(
                        ps[:],
                        _r(w_sb["wqT"][:, 256 * kt + 128 * m:256 * kt + 128 * m + 128]),
                        _r(x_sb[:, NPX * kt + QOFF + 512 * n:
                                NPX * kt + QOFF + 512 * n + 512]),
                        start=(kt == 0), stop=(kt == 1))
                nc.vector.tensor_scalar_add(q_sb[m][:, 512 * n:512 * n + 512],
                                            ps[:], bq2_sb[:, m:m + 1])
            for n in range(NPX // 512):
                ps = _ptile([128, 512], f"psk{m}_{n}")
                for kt in range(2):
                    nc.tensor.matmul(
                        ps[:],
                        _r(w_sb["wkT"][:, 256 * kt + 128 * m:256 * kt + 128 * m + 128]),
                        _r(x_sb[:, NPX * kt + 512 * n:NPX * kt + 512 * n + 512]),
                        start=(kt == 0), stop=(kt == 1))
                nc.gpsimd.tensor_copy(k_sb[m][:, 512 * n:512 * n + 512], ps[:])

        for t in range(NKB):
            ps = _ptile([128, 256], f"psv{t}")
            for kt in range(2):
                lhsT = x_sb[:, NPX * kt + 128 * t:NPX * kt + 128 * t + 128]
                nc.tensor.matmul(ps[:],
                                 _r(lhsT), _r(w_sb["wvT"][:, 256 * kt:256 * kt + 256]),
                                 start=(kt == 0), stop=(kt == 1))
            nc.gpsimd.tensor_copy(v2_sb[:, 256 * t:256 * t + 256], ps[:])

        # ---- attention: pipelined over (g, kb) ----
        msk = masks_sb[:]
        av_tiles = {}
        z_tiles = {}

        def emit_S(g, kb):
            qc0 = _qcol0(kb)
            halves = []
            for h2 in range(2):
                spsum = sp.tile([128, 1024], F32, tag="s", name=f"s{g}_{kb}_{h2}")
                for j in range(2):
                    hh = 2 * h2 + j
                    nc.tensor.matmul(
                        spsum[:, 512 * j:512 * j + 512],
                        _r(k_sb[g][32 * hh:32 * hh + 32, 128 * kb:128 * kb + 128]),
                        _r(q_sb[g][32 * hh:32 * hh + 32, qc0:qc0 + 512]),
                        start=True, stop=True, tile_position=(32 * hh, 0))
                halves.append(spsum)
            return halves

        def emit_expmask(g, kb, halves):
            qc0 = _qcol0(kb)
            msks = []
            for h2 in range(2):
                exp_t = wp.tile([128, 1024], BF16, tag=f"exp{h2}")
                nc.scalar.activation(exp_t[:], halves[h2][:],
                                     mybir.ActivationFunctionType.Exp)
                msk_t = wp.tile([128, 1024], BF16, tag=f"msk{h2}")
                mask_bcast = bass.AP(msk.tensor, msk.offset + 512 * kb,
                                     [[NKB * 512, 128], [0, 2], [1, 512]])
                nc.vector.tensor_mul(msk_t[:], exp_t[:], mask_bcast)
                msks.append(msk_t)
            return msks

        def emit_zav(g, kb, msks):
            qc0 = _qcol0(kb)
            for r in range(4):
                if kb not in contribs[r]:
                    continue
                if kb == first_kb[r]:
                    avt = avp.tile([128, 512], F32, tag="av", name=f"av{g}_{r}")
                    av_tiles[(g, r)] = avt
                    zt = zp.tile([128, 512], F32, tag="z", name=f"z{g}_{r}")
                    z_tiles[(g, r)] = zt
                    nc.tensor.matmul(avt[:], zrow_sb[0:1, 0:128],
                                     zrow_sb[0:1, 0:512], start=True, stop=False)
                    nc.tensor.matmul(zt[:], onerow_sb[0:1, 0:128],
                                     epsrow_sb[0:1, 0:512], start=True, stop=False)
                avt = av_tiles[(g, r)]
                zt = z_tiles[(g, r)]
                c0 = max(512 * r, qc0)
                c1 = min(512 * r + 512, qc0 + 512)
                last = (kb == last_kb[r])
                for hh in range(4):
                    mt = msks[hh // 2]
                    mslc = mt[:, 512 * (hh % 2) + c0 - qc0:512 * (hh % 2) + c1 - qc0]
                    nc.tensor.matmul(
                        zt[32 * hh:32 * hh + 1, c0 - 512 * r:c1 - 512 * r],
                        ones_sb[:, 0:1], mslc,
                        start=False, stop=(last and hh == 3),
                        tile_position=(0, 32 * hh))
                    nc.tensor.matmul(
                        avt[32 * hh:32 * hh + 32, c0 - 512 * r:c1 - 512 * r],
                        v2_sb[:, 256 * kb + 32 * (4 * g + hh):
                              256 * kb + 32 * (4 * g + hh) + 32],
                        mslc,
                        start=False, stop=(last and hh == 3),
                        tile_position=(0, 32 * hh))
                if last:
                    zr_t = wp.tile([128, 512], F32, tag="zr")
                    nc.vector.reciprocal(zr_t[:], zt[:])
                    zrp = sp.tile([128, 512], F32, tag="s", name=f"zrp{g}_{r}")
                    nc.tensor.matmul(zrp[:], _r(repmat_sb[:]), _r(zr_t[:]),
                                     start=True, stop=True)
                    zrep_t = wp.tile([128, 512], F32, tag="zrep")
                    nc.vector.tensor_copy(zrep_t[:], zrp[:])
                    nc.vector.tensor_mul(
                        avn_sb[:, NQ * g + 512 * r:NQ * g + 512 * r + 512],
                        avt[:], zrep_t[:])
                    del av_tiles[(g, r)]
                    del z_tiles[(g, r)]

        prev = None
        for g in range(2):
            for kb in range(NKB):
                if prev is not None:
                    pm = emit_expmask(*prev)
                halves = emit_S(g, kb)
                if prev is not None:
                    emit_zav(prev[0], prev[1], pm)
                prev = (g, kb, halves)
        pm = emit_expmask(*prev)
        emit_zav(prev[0], prev[1], pm)

        # ---- output projection ----
        for r in range(4):
            for m in range(2):
                ps = _ptile([128, 512], f"psop{r}_{m}")
                for g in range(2):
                    nc.tensor.matmul(
                        ps[:],
                        _r(w_sb["woT"][:, 256 * g + 128 * m:256 * g + 128 * m + 128]),
                        _r(avn_sb[:, NQ * g + 512 * r:NQ * g + 512 * r + 512]),
                        start=(g == 0), stop=(g == 1))
                ot = wp.tile([128, 512], F32, tag="zr", name=f"ot{r}_{m}")
                nc.vector.tensor_scalar_add(ot[:], ps[:], bo2_sb[:, m:m + 1])
                nc.sync.dma_start(y[128 * m:128 * m + 128, 512 * r:512 * r + 512],
                                  ot[:])

    nc.compile()
    return nc


def _host_inputs(inputs):
    query = np.ascontiguousarray(inputs["query"], np.float32)
    wq = np.asarray(inputs["wq"], np.float32)
    bq = np.asarray(inputs["bq"], np.float32)
    wk = np.asarray(inputs["wk"], np.float32)
    wv = np.asarray(inputs["wv"], np.float32)
    bv = np.asarray(inputs["bv"], np.float32)
    wo = np.asarray(inputs["wo"], np.float32)
    bo = np.asarray(inputs["bo"], np.float32)

    wqT = np.ascontiguousarray((wq * SCALE).T)
    wkT = np.ascontiguousarray(wk.T)
    wvT = np.ascontiguousarray(wv.T)
    woT = np.ascontiguousarray(wo.T)
    bq2 = np.ascontiguousarray((SCALE * bq).reshape(2, 128).T)
    bo2v = (wo @ bv + bo).astype(np.float32)
    bo2 = np.ascontiguousarray(bo2v.reshape(2, 128).T)  # [128, 2] col m
    repmat = np.zeros((128, 128), np.float32)
    for pout in range(128):
        repmat[32 * (pout // 32), pout] = 1.0

    in_maps = []
    for core in range(8):
        b, R0 = _core_geom(core)
        xpad = np.zeros((DIM, KVR, 64), np.float32)
        lo, hi = R0 - 4, R0 + 36
        slo, shi = max(lo, 0), min(hi, H)
        xpad[:, slo - lo:shi - lo, :] = query[b][:, slo:shi, :]
        masks = np.concatenate([_build_mask(R0, kb) for kb in range(NKB)],
                               axis=1)
        in_maps.append({
            "x": np.ascontiguousarray(xpad.reshape(DIM, NPX)),
            "wqT": wqT, "wkT": wkT, "wvT": wvT, "woT": woT,
            "masks": np.ascontiguousarray(masks.astype(ml_dtypes.bfloat16)),
            "repmat": repmat, "bq2": bq2, "bo2": bo2,
        })
    return in_maps


_nc_cache = None


def kernel(**inputs):
    global _nc_cache
    in_maps = _host_inputs(inputs)
    if _nc_cache is None:
        _nc_cache = _build_program()
    res = run_bass_kernel_spmd(_nc_cache, in_maps, core_ids=list(range(8)))
    out = np.zeros((B, DIM, H, W), np.float32)
    for core in range(8):
        b, R0 = _core_geom(core)
        out[b][:, R0:R0 + 32, :] = res.results[core]["y"].reshape(DIM, 32, 64)
    return out
